# revision 40
# baseline (speedup 1.0000x reference)
"""Trainium2 Bass kernel for nn_Attention_90967407330064.

Dense single-head spatial attention over x:[B,C,H,W] with 1x1-conv QKV:
  q = Wq@x+bq [B,64,N], k = Wk@x+bk, v = Wv@x+bv [B,256,N], N=H*W=4096
  out = v @ softmax(qT k / sqrt(N)) + x

Sharding: data-parallel over batch B=16 across 8 cores (2 batches/core).

Per-batch device algorithm (all layouts chosen so no transposes of the
big NxN matrix are ever needed):
  - x loaded as 2 c-tiles [128, 4096] fp32.
  - QKV projections on PE in float32r (full rate). v is produced directly
    TRANSPOSED (vT[n,c] = sum_c' x[c',n] WvT[c',c]) so it can serve as the
    stationary operand of the output matmul.
  - Scores computed transposed: S_T[j,i] = k[:,j].q[:,i] (contract DA=64).
  - exp on ScalarE with scale=1/sqrt(N) folded in; no max subtraction
    (scores/64 ~ N(0, .125), bounded; exp is safe) -> E_T bf16.
  - out_unnorm[c,i] = sum_j vT[j,c] E_T[j,i] accumulated in PSUM over j.
    Softmax denominator D[i] = sum_j E_T[j,i] via an all-ones [128,128]
    stationary matmul into a parallel PSUM bank (replicates D across all
    partitions for free).
  - tail: recipD = reciprocal_approx_fast(D); out = psum*recipD + (x+bv).
"""

import math
from contextlib import ExitStack

import numpy as np

import concourse.bass as bass
import concourse.tile as tile
from concourse import bacc, mybir
from concourse.bass import ds, ts
from concourse.masks import make_identity

dt = mybir.dt

# Problem constants (hardcoded per harness contract).
B, C, H, W = 16, 256, 64, 64
DA = 64
N = H * W
N_CORES = 8
BPC = B // N_CORES  # batches per core

P = 128  # partitions
IC = 512  # i-chunk (psum bank width in fp32)


def build_nc(bpc=BPC, c_dim=C, n_dim=N, da=DA, ic=IC, repeat=1):
    """Build the per-core Bass kernel. Parameterized so a small config can be
    validated in CoreSim. repeat>1 re-runs the whole workload (idempotent) for
    dispatch-overhead-free wall-clock timing via the slope over repeat."""
    assert c_dim % P == 0 and n_dim % ic == 0 and n_dim % P == 0
    CT = c_dim // P  # c-tiles (2)
    KC = c_dim // P  # contraction chunks over c' (2)
    NIC = n_dim // ic  # i-chunks (8)
    NJT = n_dim // P  # j-tiles (32)
    assert NJT % 2 == 0
    inv_sqrt_n = 1.0 / math.sqrt(float(n_dim))

    nc = bacc.Bacc(
        "TRN2", target_bir_lowering=False, debug=False, enable_asserts=False
    )
    f32, bf16, f8 = dt.float32, dt.bfloat16, dt.float8e4

    x_d = nc.dram_tensor("x", [bpc, c_dim, n_dim], f32, kind="ExternalInput").ap()
    wq_d = nc.dram_tensor("Wq", [da, c_dim], f32, kind="ExternalInput").ap()
    bq_d = nc.dram_tensor("bq", [da], f32, kind="ExternalInput").ap()
    wk_d = nc.dram_tensor("Wk", [da, c_dim], f32, kind="ExternalInput").ap()
    bk_d = nc.dram_tensor("bk", [da], f32, kind="ExternalInput").ap()
    wv_d = nc.dram_tensor("Wv", [c_dim, c_dim], f32, kind="ExternalInput").ap()
    bv_d = nc.dram_tensor("bv", [c_dim], f32, kind="ExternalInput").ap()
    out_d = nc.dram_tensor("out", [bpc, c_dim, n_dim], f32, kind="ExternalOutput").ap()

    with tile.TileContext(nc) as tc, ExitStack() as ctx:
        consts = ctx.enter_context(tc.tile_pool(name="consts", bufs=1))
        xpool = ctx.enter_context(tc.tile_pool(name="xp", bufs=2))
        bigs = ctx.enter_context(tc.tile_pool(name="bigs", bufs=2))
        et_pool = ctx.enter_context(tc.tile_pool(name="et", bufs=5))
        outs = ctx.enter_context(tc.tile_pool(name="outsb", bufs=3))
        small = ctx.enter_context(tc.tile_pool(name="small", bufs=2))
        # All phase-transient PSUM tiles share one tag -> 2 slots x 2 banks.
        ps_s = ctx.enter_context(tc.tile_pool(name="ps_s", bufs=2, space="PSUM"))
        ps_out = ctx.enter_context(tc.tile_pool(name="ps_out", bufs=1, space="PSUM"))
        ps_d = ctx.enter_context(tc.tile_pool(name="ps_d", bufs=2, space="PSUM"))

        # --- constants / weights (once per kernel) ---
        ident = consts.tile([P, P], f32)
        make_identity(nc, ident)
        # all-ones stationary operand for the denominator matmul, fp8
        # DoubleRow layout [P, 2, P]
        ones_f8 = consts.tile([P, 2 * P], f8)
        nc.vector.memset(ones_f8, 1.0)
        ones_f8_v = ones_f8.rearrange("p (two m) -> p two m", two=2)

        wq_sb = consts.tile([da, c_dim], f32, tag="wq")
        nc.sync.dma_start(wq_sb, wq_d)
        wk_sb = consts.tile([da, c_dim], f32, tag="wk")
        nc.sync.dma_start(wk_sb, wk_d)
        wv_sb = []
        for ct in range(CT):
            t = consts.tile([P, c_dim], f32, tag=f"wv{ct}")
            nc.sync.dma_start(t, wv_d[ts(ct, P), :])
            wv_sb.append(t)

        bq_sb = consts.tile([da, 1], f32, tag="bq")
        nc.sync.dma_start(bq_sb, bq_d.rearrange("(a o) -> a o", o=1))
        bk_sb = consts.tile([da, 1], f32, tag="bk")
        nc.sync.dma_start(bk_sb, bk_d.rearrange("(a o) -> a o", o=1))
        bv_sb = consts.tile([P, CT], f32, tag="bv")
        nc.sync.dma_start(bv_sb, bv_d.rearrange("(ct p) -> p ct", p=P))

        # PE warmup: dummy matmuls on the identity keep the HAM activity
        # window busy while the first x DMA lands, so real matmuls start at
        # the warm 2.4 GHz clock instead of 1.2 GHz.
        warm_ps = ps_s.tile([P, ic], f32, tag="ps", name="warm_ps")
        for _ in range(40):
            nc.tensor.matmul(
                warm_ps[:, :P], ident, ident, start=True, stop=True
            )

        # Transposed weights via PE transpose: wqT/wkT[kc] = [128, da],
        # wvT[kc] = [128, c_dim] (= Wv[:, kc-cols].T laid out c' x c).
        # Stored fp8 in DoubleRow layout (kc = the middle pair dim) so each
        # QKV projection needs a single K=256 DoubleRow matmul.
        wqT = consts.tile([P, KC, da], f8, tag="wqT")
        wkT = consts.tile([P, KC, da], f8, tag="wkT")
        wvT = consts.tile([P, KC, c_dim], f8, tag="wvT")
        for kc in range(KC):
            pt = ps_s.tile([P, P], f32, tag="ps")
            nc.tensor.transpose(pt[:, :da], wq_sb[:, ts(kc, P)], ident[:da, :da])
            nc.scalar.copy(wqT[:, kc, :], pt[:, :da])
            pt2 = ps_s.tile([P, P], f32, tag="ps")
            nc.tensor.transpose(pt2[:, :da], wk_sb[:, ts(kc, P)], ident[:da, :da])
            nc.scalar.copy(wkT[:, kc, :], pt2[:, :da])
            for ct in range(CT):
                pt3 = ps_s.tile([P, P], f32, tag="ps")
                nc.tensor.transpose(pt3, wv_sb[ct][:, ts(kc, P)], ident)
                nc.scalar.copy(wvT[:, kc, ts(ct, P)], pt3)

        for b in [b for _ in range(repeat) for b in range(bpc)]:
            # --- phase 1: load x ---
            x_sb = []
            for ct in range(CT):
                t = xpool.tile([P, n_dim], f32, tag=f"x{ct}", name=f"x{ct}")
                for half in range(4):
                    nc.sync.dma_start(
                        t[:, ts(half, n_dim // 4)],
                        x_d[b, ts(ct, P), ts(half, n_dim // 4)],
                    )
                x_sb.append(t)
            # fp8 copy of x in DoubleRow layout [128, KC, n] feeding the QKV
            # projection matmuls, split into chunks so they start early
            x_f8 = bigs.tile([P, KC, n_dim], f8, tag="xf8", name="xf8")
            for ct in range(CT):
                for n_i in range(NIC):
                    nc.vector.tensor_copy(
                        x_f8[:, ct, ts(n_i, ic)], x_sb[ct][:, ts(n_i, ic)]
                    )

            # --- phase 2: q, k [128, n] bf16, replicated into both partition
            # halves so the scores matmuls can be 2-way row-packed (K=64 each
            # at row groups 0 and 64). ---
            q_sb = bigs.tile([P, n_dim], bf16, tag="q")
            k_sb = bigs.tile([P, n_dim], bf16, tag="k")
            for n_i in range(NIC):
                pq = ps_s.tile([da, ic], f32, tag="ps")
                nc.tensor.matmul(
                    pq,
                    wqT,
                    x_f8[:, :, ts(n_i, ic)],
                    start=True,
                    stop=True,
                    perf_mode=mybir.MatmulPerfMode.DoubleRow,
                )
                nc.vector.tensor_scalar_add(q_sb[:da, ts(n_i, ic)], pq, bq_sb)
                nc.vector.tensor_copy(q_sb[da:, ts(n_i, ic)], q_sb[:da, ts(n_i, ic)])
                pk = ps_s.tile([da, ic], f32, tag="ps")
                nc.tensor.matmul(
                    pk,
                    wkT,
                    x_f8[:, :, ts(n_i, ic)],
                    start=True,
                    stop=True,
                    perf_mode=mybir.MatmulPerfMode.DoubleRow,
                )
                nc.vector.tensor_scalar_add(k_sb[:da, ts(n_i, ic)], pk, bk_sb)
                nc.vector.tensor_copy(k_sb[da:, ts(n_i, ic)], k_sb[:da, ts(n_i, ic)])

            # --- phase 3: vT [n, c] fp8, stored DoubleRow-ready as
            # [128, NJT/2, 2, c] (middle dims: j-tile pair, pair member) ---
            vT_sb = bigs.tile([P, NJT // 2, 2, c_dim], f8, tag="vT")
            for t_j in range(NJT):
                pv = ps_s.tile([P, c_dim], f32, tag="ps")
                nc.tensor.matmul(
                    pv,
                    x_f8[:, :, ts(t_j, P)],
                    wvT,
                    start=True,
                    stop=True,
                    perf_mode=mybir.MatmulPerfMode.DoubleRow,
                )
                nc.vector.tensor_copy(vT_sb[:, t_j // 2, t_j % 2, :], pv)

            # --- phase 4: attention main loop ---
            for i_c in range(NIC):
                po = [
                    ps_out.tile([P, ic], f32, tag=f"o{c0}", name=f"po{c0}")
                    for c0 in range(CT)
                ]
                pd = ps_d.tile([P, ic], f32, tag="d")
                # Software-pipelined emission: PE engine queues are strict
                # FIFO, so out-matmuls are emitted one pair behind the score
                # matmuls (hiding the exp latency behind queued PE work) and
                # the denominator matmul two pairs behind (hiding the DVE
                # pair-sum latency).
                NP = NJT // 2
                NQ = NP // 2
                ets = [None] * NP
                esums = [None] * NQ

                def emit_scores(jp):
                    # two K=64 score matmuls packed into row groups 0 / 64,
                    # outputs to the two banks of one [128, 1024] psum tile
                    ps_pair = ps_s.tile([P, 2 * ic], f32, tag="ps", name="ps_pair")
                    nc.tensor.matmul(
                        ps_pair[:, ts(0, ic)],
                        k_sb[:da, ts(2 * jp, P)],
                        q_sb[:da, ts(i_c, ic)],
                        start=True,
                        stop=True,
                        tile_position=(0, 0),
                    )
                    nc.tensor.matmul(
                        ps_pair[:, ts(1, ic)],
                        k_sb[da:, ts(2 * jp + 1, P)],
                        q_sb[da:, ts(i_c, ic)],
                        start=True,
                        stop=True,
                        tile_position=(da, 0),
                    )
                    # exp -> fp8 E^T, already in DoubleRow [P, 2, ic] layout
                    et = et_pool.tile([P, 2 * ic], f8, tag="et", name="et")
                    nc.scalar.activation(
                        et, ps_pair, mybir.ActivationFunctionType.Exp, scale=inv_sqrt_n
                    )
                    ets[jp] = et

                def emit_out(jp):
                    for c0 in range(CT):
                        nc.tensor.matmul(
                            po[c0],
                            vT_sb[:, jp, :, ts(c0, P)],
                            ets[jp].rearrange("p (two f) -> p two f", two=2),
                            start=(jp == 0),
                            stop=(jp == NP - 1),
                            perf_mode=mybir.MatmulPerfMode.DoubleRow,
                            skip_group_check=True,
                        )

                def emit_d(jp):
                    nc.tensor.matmul(
                        pd,
                        ones_f8_v,
                        ets[jp].rearrange("p (two f) -> p two f", two=2),
                        start=(jp == 0),
                        stop=(jp == NP - 1),
                        perf_mode=mybir.MatmulPerfMode.DoubleRow,
                        skip_group_check=True,
                    )

                for jp in range(NP):
                    emit_scores(jp)
                    if jp >= 2:
                        emit_out(jp - 2)
                        emit_d(jp - 2)
                for jp in (NP - 2, NP - 1):
                    emit_out(jp)
                    emit_d(jp)
                # tail: out = po*recipD + bv + x
                rd = small.tile([P, ic], f32, tag="rd")
                nc.vector.reciprocal_approx_fast(rd, pd)
                for c0 in range(CT):
                    ob = outs.tile([P, ic], f32, tag="ob")
                    nc.vector.tensor_mul(ob, po[c0], rd)
                    nc.vector.scalar_tensor_tensor(
                        ob,
                        ob,
                        bv_sb[:, ds(c0, 1)],
                        x_sb[c0][:, ts(i_c, ic)],
                        mybir.AluOpType.add,
                        mybir.AluOpType.add,
                    )
                    nc.sync.dma_start(out_d[b, ts(c0, P), ts(i_c, ic)], ob)

    nc.compile()
    return nc


_NC_CACHE = None


def get_nc():
    global _NC_CACHE
    if _NC_CACHE is None:
        _NC_CACHE = build_nc()
    return _NC_CACHE


def make_in_maps(inputs) -> list:
    x = np.ascontiguousarray(np.asarray(inputs["x"], dtype=np.float32)).reshape(
        B, C, N
    )
    w = {
        name: np.ascontiguousarray(np.asarray(inputs[name], dtype=np.float32))
        for name in ("Wq", "bq", "Wk", "bk", "Wv", "bv")
    }
    in_maps = []
    for c in range(N_CORES):
        m = {"x": np.ascontiguousarray(x[c * BPC : (c + 1) * BPC])}
        m.update(w)
        in_maps.append(m)
    return in_maps


def kernel(**inputs) -> np.ndarray:
    from concourse.bass_utils import run_bass_kernel_spmd

    res = run_bass_kernel_spmd(
        get_nc(), make_in_maps(inputs), core_ids=list(range(N_CORES))
    )
    out = np.concatenate([r["out"] for r in res.results], axis=0)
    return out.reshape(B, C, H, W).astype(np.float32)


# revision 42
# speedup vs baseline: 1.0021x; 1.0021x over previous
"""Trainium2 Bass kernel for nn_Attention_90967407330064.

Dense single-head spatial attention over x:[B,C,H,W] with 1x1-conv QKV:
  q = Wq@x+bq [B,64,N], k = Wk@x+bk, v = Wv@x+bv [B,256,N], N=H*W=4096
  out = v @ softmax(qT k / sqrt(N)) + x

Sharding: data-parallel over batch B=16 across 8 cores (2 batches/core).

Per-batch device algorithm (all layouts chosen so no transposes of the
big NxN matrix are ever needed):
  - x loaded as 2 c-tiles [128, 4096] fp32; bf16 copy feeds the QKV matmuls.
  - QKV projections on PE in bf16. v is produced directly TRANSPOSED
    (vT[n,c] = sum_c' x[c',n] WvT[c',c]) into fp8 DoubleRow layout so it
    can serve as the stationary operand of the output matmul.
  - Scores computed transposed: S_T[j,i] = k[:,j].q[:,i] (contract DA=64),
    two j-tiles 2-way row-packed into PE row groups 0/64 concurrently.
  - exp on ScalarE with scale=1/sqrt(N) folded in; no max subtraction
    (scores/64 ~ N(0, .125), bounded; exp is safe) -> E_T fp8e4m3 written
    straight into DoubleRow [P, 2, 512] layout.
  - out_unnorm[c,i] = sum_j vT[j,c] E_T[j,i] accumulated in PSUM over
    j-tile pairs with fp8 DoubleRow matmuls (K=256/instruction). Softmax
    denominator D[i] = sum_j E_T[j,i] via an all-ones DoubleRow stationary
    matmul into a parallel PSUM bank (replicates D across all partitions).
    Emission is software-pipelined (out/D trail the scores by 2 pairs) to
    keep the strict-FIFO PE queue from head-of-line blocking on exp.
  - tail: recipD = reciprocal_approx_fast(D); out = po*recipD + bv + x.

Measured on 8 trn2 cores: ~395 us HW exec, rel l2 error ~6e-4 vs fp32 ref.
"""

import math
from contextlib import ExitStack

import numpy as np

import concourse.bass as bass
import concourse.tile as tile
from concourse import bacc, mybir
from concourse.bass import ds, ts
from concourse.masks import make_identity

dt = mybir.dt

# Problem constants (hardcoded per harness contract).
B, C, H, W = 16, 256, 64, 64
DA = 64
N = H * W
N_CORES = 8
BPC = B // N_CORES  # batches per core

P = 128  # partitions
IC = 512  # i-chunk (psum bank width in fp32)


def build_nc(bpc=BPC, c_dim=C, n_dim=N, da=DA, ic=IC, repeat=1):
    """Build the per-core Bass kernel. Parameterized so a small config can be
    validated in CoreSim. repeat>1 re-runs the whole workload (idempotent) for
    dispatch-overhead-free wall-clock timing via the slope over repeat."""
    assert c_dim % P == 0 and n_dim % ic == 0 and n_dim % P == 0
    CT = c_dim // P  # c-tiles (2)
    KC = c_dim // P  # contraction chunks over c' (2)
    NIC = n_dim // ic  # i-chunks (8)
    NJT = n_dim // P  # j-tiles (32)
    assert NJT % 2 == 0
    inv_sqrt_n = 1.0 / math.sqrt(float(n_dim))

    nc = bacc.Bacc(
        "TRN2", target_bir_lowering=False, debug=False, enable_asserts=False
    )
    f32, bf16, f8 = dt.float32, dt.bfloat16, dt.float8e4

    x_d = nc.dram_tensor("x", [bpc, c_dim, n_dim], f32, kind="ExternalInput").ap()
    wq_d = nc.dram_tensor("Wq", [da, c_dim], f32, kind="ExternalInput").ap()
    bq_d = nc.dram_tensor("bq", [da], f32, kind="ExternalInput").ap()
    wk_d = nc.dram_tensor("Wk", [da, c_dim], f32, kind="ExternalInput").ap()
    bk_d = nc.dram_tensor("bk", [da], f32, kind="ExternalInput").ap()
    wv_d = nc.dram_tensor("Wv", [c_dim, c_dim], f32, kind="ExternalInput").ap()
    bv_d = nc.dram_tensor("bv", [c_dim], f32, kind="ExternalInput").ap()
    out_d = nc.dram_tensor("out", [bpc, c_dim, n_dim], f32, kind="ExternalOutput").ap()

    with tile.TileContext(nc) as tc, ExitStack() as ctx:
        consts = ctx.enter_context(tc.tile_pool(name="consts", bufs=1))
        xpool = ctx.enter_context(tc.tile_pool(name="xp", bufs=2))
        bigs = ctx.enter_context(tc.tile_pool(name="bigs", bufs=2))
        et_pool = ctx.enter_context(tc.tile_pool(name="et", bufs=5))
        outs = ctx.enter_context(tc.tile_pool(name="outsb", bufs=3))
        small = ctx.enter_context(tc.tile_pool(name="small", bufs=2))
        # All phase-transient PSUM tiles share one tag -> 2 slots x 2 banks.
        ps_s = ctx.enter_context(tc.tile_pool(name="ps_s", bufs=2, space="PSUM"))
        ps_out = ctx.enter_context(tc.tile_pool(name="ps_out", bufs=1, space="PSUM"))
        ps_d = ctx.enter_context(tc.tile_pool(name="ps_d", bufs=2, space="PSUM"))

        # --- constants / weights (once per kernel) ---
        ident = consts.tile([P, P], f32)
        make_identity(nc, ident)
        # all-ones stationary operand for the denominator matmul, fp8
        # DoubleRow layout [P, 2, P]
        ones_f8 = consts.tile([P, 2 * P], f8)
        nc.vector.memset(ones_f8, 1.0)
        ones_f8_v = ones_f8.rearrange("p (two m) -> p two m", two=2)

        wq_sb = consts.tile([da, c_dim], f32, tag="wq")
        nc.sync.dma_start(wq_sb, wq_d)
        wk_sb = consts.tile([da, c_dim], f32, tag="wk")
        nc.sync.dma_start(wk_sb, wk_d)
        wv_sb = []
        for ct in range(CT):
            t = consts.tile([P, c_dim], f32, tag=f"wv{ct}")
            nc.sync.dma_start(t, wv_d[ts(ct, P), :])
            wv_sb.append(t)

        bq_sb = consts.tile([da, 1], f32, tag="bq")
        nc.sync.dma_start(bq_sb, bq_d.rearrange("(a o) -> a o", o=1))
        bk_sb = consts.tile([da, 1], f32, tag="bk")
        nc.sync.dma_start(bk_sb, bk_d.rearrange("(a o) -> a o", o=1))
        bv_sb = consts.tile([P, CT], f32, tag="bv")
        nc.sync.dma_start(bv_sb, bv_d.rearrange("(ct p) -> p ct", p=P))

        # PE warmup: dummy matmuls on the identity keep the HAM activity
        # window busy while the first x DMA lands, so real matmuls start at
        # the warm 2.4 GHz clock instead of 1.2 GHz.
        warm_ps = ps_s.tile([P, ic], f32, tag="ps", name="warm_ps")
        for _ in range(24):
            nc.tensor.matmul(
                warm_ps[:, :P], ident, ident, start=True, stop=True
            )

        # Transposed weights via PE transpose: wqT/wkT[kc] = [128, da],
        # wvT[kc] = [128, c_dim] (= Wv[:, kc-cols].T laid out c' x c).
        # Stored bf16 (the PSUM->SBUF copy converts) for full-rate matmuls.
        wqT = consts.tile([P, KC, da], bf16, tag="wqT")
        wkT = consts.tile([P, KC, da], bf16, tag="wkT")
        wvT = consts.tile([P, KC, c_dim], bf16, tag="wvT")
        for kc in range(KC):
            pt = ps_s.tile([P, P], f32, tag="ps")
            nc.tensor.transpose(pt[:, :da], wq_sb[:, ts(kc, P)], ident[:da, :da])
            nc.scalar.copy(wqT[:, kc, :], pt[:, :da])
            pt2 = ps_s.tile([P, P], f32, tag="ps")
            nc.tensor.transpose(pt2[:, :da], wk_sb[:, ts(kc, P)], ident[:da, :da])
            nc.scalar.copy(wkT[:, kc, :], pt2[:, :da])
            for ct in range(CT):
                pt3 = ps_s.tile([P, P], f32, tag="ps")
                nc.tensor.transpose(pt3, wv_sb[ct][:, ts(kc, P)], ident)
                nc.scalar.copy(wvT[:, kc, ts(ct, P)], pt3)

        for b in [b for _ in range(repeat) for b in range(bpc)]:
            # --- phase 1: load x ---
            x_sb = []
            for ct in range(CT):
                t = xpool.tile([P, n_dim], f32, tag=f"x{ct}", name=f"x{ct}")
                for half in range(4):
                    nc.sync.dma_start(
                        t[:, ts(half, n_dim // 4)],
                        x_d[b, ts(ct, P), ts(half, n_dim // 4)],
                    )
                x_sb.append(t)
            # bf16 copy of x feeding the QKV projection matmuls, split into
            # chunks so the first projection matmuls start early
            x_bf = []
            for ct in range(CT):
                t = bigs.tile([P, n_dim], bf16, tag=f"xbf{ct}", name=f"xbf{ct}")
                for n_i in range(NIC):
                    nc.vector.tensor_copy(
                        t[:, ts(n_i, ic)], x_sb[ct][:, ts(n_i, ic)]
                    )
                x_bf.append(t)

            # --- phase 2: q, k [128, n] bf16, replicated into both partition
            # halves so the scores matmuls can be 2-way row-packed (K=64 each
            # at row groups 0 and 64). ---
            q_sb = bigs.tile([P, n_dim], bf16, tag="q")
            k_sb = bigs.tile([P, n_dim], bf16, tag="k")
            for n_i in range(NIC):
                pq = ps_s.tile([da, ic], f32, tag="ps")
                for kc in range(KC):
                    nc.tensor.matmul(
                        pq,
                        wqT[:, kc, :],
                        x_bf[kc][:, ts(n_i, ic)],
                        start=(kc == 0),
                        stop=(kc == KC - 1),
                    )
                nc.vector.tensor_scalar_add(q_sb[:da, ts(n_i, ic)], pq, bq_sb)
                nc.vector.tensor_copy(q_sb[da:, ts(n_i, ic)], q_sb[:da, ts(n_i, ic)])
                pk = ps_s.tile([da, ic], f32, tag="ps")
                for kc in range(KC):
                    nc.tensor.matmul(
                        pk,
                        wkT[:, kc, :],
                        x_bf[kc][:, ts(n_i, ic)],
                        start=(kc == 0),
                        stop=(kc == KC - 1),
                    )
                nc.vector.tensor_scalar_add(k_sb[:da, ts(n_i, ic)], pk, bk_sb)
                nc.vector.tensor_copy(k_sb[da:, ts(n_i, ic)], k_sb[:da, ts(n_i, ic)])

            # --- phase 3: vT [n, c] fp8, stored DoubleRow-ready as
            # [128, NJT/2, 2, c] (middle dims: j-tile pair, pair member) ---
            vT_sb = bigs.tile([P, NJT // 2, 2, c_dim], f8, tag="vT")
            for t_j in range(NJT):
                pv = ps_s.tile([P, c_dim], f32, tag="ps")
                for kc in range(KC):
                    nc.tensor.matmul(
                        pv,
                        x_bf[kc][:, ts(t_j, P)],
                        wvT[:, kc, :],
                        start=(kc == 0),
                        stop=(kc == KC - 1),
                    )
                nc.vector.tensor_copy(vT_sb[:, t_j // 2, t_j % 2, :], pv)

            # --- phase 4: attention main loop ---
            for i_c in range(NIC):
                po = [
                    ps_out.tile([P, ic], f32, tag=f"o{c0}", name=f"po{c0}")
                    for c0 in range(CT)
                ]
                pd = ps_d.tile([P, ic], f32, tag="d")
                # Software-pipelined emission: PE engine queues are strict
                # FIFO, so out-matmuls are emitted one pair behind the score
                # matmuls (hiding the exp latency behind queued PE work) and
                # the denominator matmul two pairs behind (hiding the DVE
                # pair-sum latency).
                NP = NJT // 2
                NQ = NP // 2
                ets = [None] * NP
                esums = [None] * NQ

                def emit_scores(jp):
                    # two K=64 score matmuls packed into row groups 0 / 64,
                    # outputs to the two banks of one [128, 1024] psum tile
                    ps_pair = ps_s.tile([P, 2 * ic], f32, tag="ps", name="ps_pair")
                    nc.tensor.matmul(
                        ps_pair[:, ts(0, ic)],
                        k_sb[:da, ts(2 * jp, P)],
                        q_sb[:da, ts(i_c, ic)],
                        start=True,
                        stop=True,
                        tile_position=(0, 0),
                    )
                    nc.tensor.matmul(
                        ps_pair[:, ts(1, ic)],
                        k_sb[da:, ts(2 * jp + 1, P)],
                        q_sb[da:, ts(i_c, ic)],
                        start=True,
                        stop=True,
                        tile_position=(da, 0),
                    )
                    # exp -> fp8 E^T, already in DoubleRow [P, 2, ic] layout
                    et = et_pool.tile([P, 2 * ic], f8, tag="et", name="et")
                    nc.scalar.activation(
                        et, ps_pair, mybir.ActivationFunctionType.Exp, scale=inv_sqrt_n
                    )
                    ets[jp] = et

                def emit_out(jp):
                    for c0 in range(CT):
                        nc.tensor.matmul(
                            po[c0],
                            vT_sb[:, jp, :, ts(c0, P)],
                            ets[jp].rearrange("p (two f) -> p two f", two=2),
                            start=(jp == 0),
                            stop=(jp == NP - 1),
                            perf_mode=mybir.MatmulPerfMode.DoubleRow,
                            skip_group_check=True,
                        )

                def emit_d(jp):
                    nc.tensor.matmul(
                        pd,
                        ones_f8_v,
                        ets[jp].rearrange("p (two f) -> p two f", two=2),
                        start=(jp == 0),
                        stop=(jp == NP - 1),
                        perf_mode=mybir.MatmulPerfMode.DoubleRow,
                        skip_group_check=True,
                    )

                for jp in range(NP):
                    emit_scores(jp)
                    if jp >= 2:
                        emit_out(jp - 2)
                        emit_d(jp - 2)
                for jp in (NP - 2, NP - 1):
                    emit_out(jp)
                    emit_d(jp)
                # tail: out = po*recipD + bv + x
                rd = small.tile([P, ic], f32, tag="rd")
                nc.vector.reciprocal_approx_fast(rd, pd)
                for c0 in range(CT):
                    ob = outs.tile([P, ic], f32, tag="ob")
                    nc.vector.tensor_mul(ob, po[c0], rd)
                    nc.vector.scalar_tensor_tensor(
                        ob,
                        ob,
                        bv_sb[:, ds(c0, 1)],
                        x_sb[c0][:, ts(i_c, ic)],
                        mybir.AluOpType.add,
                        mybir.AluOpType.add,
                    )
                    nc.sync.dma_start(out_d[b, ts(c0, P), ts(i_c, ic)], ob)

    nc.compile()
    return nc


_NC_CACHE = None


def get_nc():
    global _NC_CACHE
    if _NC_CACHE is None:
        _NC_CACHE = build_nc()
    return _NC_CACHE


def make_in_maps(inputs) -> list:
    x = np.ascontiguousarray(np.asarray(inputs["x"], dtype=np.float32)).reshape(
        B, C, N
    )
    w = {
        name: np.ascontiguousarray(np.asarray(inputs[name], dtype=np.float32))
        for name in ("Wq", "bq", "Wk", "bk", "Wv", "bv")
    }
    in_maps = []
    for c in range(N_CORES):
        m = {"x": np.ascontiguousarray(x[c * BPC : (c + 1) * BPC])}
        m.update(w)
        in_maps.append(m)
    return in_maps


def kernel(**inputs) -> np.ndarray:
    from concourse.bass_utils import run_bass_kernel_spmd

    res = run_bass_kernel_spmd(
        get_nc(), make_in_maps(inputs), core_ids=list(range(N_CORES))
    )
    out = np.concatenate([r["out"] for r in res.results], axis=0)
    return out.reshape(B, C, H, W).astype(np.float32)


# revision 44
# speedup vs baseline: 1.0096x; 1.0075x over previous
"""Trainium2 Bass kernel for nn_Attention_90967407330064.

Dense single-head spatial attention over x:[B,C,H,W] with 1x1-conv QKV:
  q = Wq@x+bq [B,64,N], k = Wk@x+bk, v = Wv@x+bv [B,256,N], N=H*W=4096
  out = v @ softmax(qT k / sqrt(N)) + x

Sharding: data-parallel over batch B=16 across 8 cores (2 batches/core).

Per-batch device algorithm (all layouts chosen so no transposes of the
big NxN matrix are ever needed):
  - x loaded as 2 c-tiles [128, 4096] fp32; bf16 copy feeds the QKV matmuls.
  - QKV projections on PE in bf16. v is produced directly TRANSPOSED
    (vT[n,c] = sum_c' x[c',n] WvT[c',c]) into fp8 DoubleRow layout so it
    can serve as the stationary operand of the output matmul.
  - Scores computed transposed: S_T[j,i] = k[:,j].q[:,i] (contract DA=64),
    two j-tiles 2-way row-packed into PE row groups 0/64 concurrently.
  - exp on ScalarE with scale=1/sqrt(N) folded in; no max subtraction
    (scores/64 ~ N(0, .125), bounded; exp is safe) -> E_T fp8e4m3 written
    straight into DoubleRow [P, 2, 512] layout.
  - out_unnorm[c,i] = sum_j vT[j,c] E_T[j,i] accumulated in PSUM over
    j-tile pairs with fp8 DoubleRow matmuls (K=256/instruction). Softmax
    denominator D[i] = sum_j E_T[j,i] via an all-ones DoubleRow stationary
    matmul into a parallel PSUM bank (replicates D across all partitions).
    Emission is software-pipelined (out/D trail the scores by 2 pairs) to
    keep the strict-FIFO PE queue from head-of-line blocking on exp.
  - tail: recipD = reciprocal_approx_fast(D); out = po*recipD + bv + x.

Measured on 8 trn2 cores: ~395 us HW exec, rel l2 error ~6e-4 vs fp32 ref.
"""

import math
from contextlib import ExitStack

import numpy as np

import concourse.bass as bass
import concourse.tile as tile
from concourse import bacc, mybir
from concourse.bass import ds, ts
from concourse.masks import make_identity

dt = mybir.dt

# Problem constants (hardcoded per harness contract).
B, C, H, W = 16, 256, 64, 64
DA = 64
N = H * W
N_CORES = 8
BPC = B // N_CORES  # batches per core

P = 128  # partitions
IC = 512  # i-chunk (psum bank width in fp32)


def build_nc(bpc=BPC, c_dim=C, n_dim=N, da=DA, ic=IC, repeat=1):
    """Build the per-core Bass kernel. Parameterized so a small config can be
    validated in CoreSim. repeat>1 re-runs the whole workload (idempotent) for
    dispatch-overhead-free wall-clock timing via the slope over repeat."""
    assert c_dim % P == 0 and n_dim % ic == 0 and n_dim % P == 0
    CT = c_dim // P  # c-tiles (2)
    KC = c_dim // P  # contraction chunks over c' (2)
    NIC = n_dim // ic  # i-chunks (8)
    NJT = n_dim // P  # j-tiles (32)
    assert NJT % 2 == 0
    inv_sqrt_n = 1.0 / math.sqrt(float(n_dim))

    nc = bacc.Bacc(
        "TRN2", target_bir_lowering=False, debug=False, enable_asserts=False
    )
    f32, bf16, f8 = dt.float32, dt.bfloat16, dt.float8e4

    x_d = nc.dram_tensor("x", [bpc, c_dim, n_dim], f32, kind="ExternalInput").ap()
    wq_d = nc.dram_tensor("Wq", [da, c_dim], f32, kind="ExternalInput").ap()
    bq_d = nc.dram_tensor("bq", [da], f32, kind="ExternalInput").ap()
    wk_d = nc.dram_tensor("Wk", [da, c_dim], f32, kind="ExternalInput").ap()
    bk_d = nc.dram_tensor("bk", [da], f32, kind="ExternalInput").ap()
    wv_d = nc.dram_tensor("Wv", [c_dim, c_dim], f32, kind="ExternalInput").ap()
    bv_d = nc.dram_tensor("bv", [c_dim], f32, kind="ExternalInput").ap()
    out_d = nc.dram_tensor("out", [bpc, c_dim, n_dim], f32, kind="ExternalOutput").ap()

    with tile.TileContext(nc) as tc, ExitStack() as ctx:
        consts = ctx.enter_context(tc.tile_pool(name="consts", bufs=1))
        xpool = ctx.enter_context(tc.tile_pool(name="xp", bufs=2))
        bigs = ctx.enter_context(tc.tile_pool(name="bigs", bufs=2))
        et_pool = ctx.enter_context(tc.tile_pool(name="et", bufs=5))
        outs = ctx.enter_context(tc.tile_pool(name="outsb", bufs=3))
        small = ctx.enter_context(tc.tile_pool(name="small", bufs=2))
        # All phase-transient PSUM tiles share one tag -> 2 slots x 2 banks.
        ps_s = ctx.enter_context(tc.tile_pool(name="ps_s", bufs=2, space="PSUM"))
        ps_out = ctx.enter_context(tc.tile_pool(name="ps_out", bufs=3, space="PSUM"))
        ps_d = ctx.enter_context(tc.tile_pool(name="ps_d", bufs=1, space="PSUM"))

        # --- constants / weights (once per kernel) ---
        ident = consts.tile([P, P], f32)
        make_identity(nc, ident)
        # all-ones stationary operand for the denominator matmul, fp8
        # DoubleRow layout [P, 2, P]
        ones_f8 = consts.tile([P, 2 * P], f8)
        nc.vector.memset(ones_f8, 1.0)
        ones_f8_v = ones_f8.rearrange("p (two m) -> p two m", two=2)

        wq_sb = consts.tile([da, c_dim], f32, tag="wq")
        nc.sync.dma_start(wq_sb, wq_d)
        wk_sb = consts.tile([da, c_dim], f32, tag="wk")
        nc.sync.dma_start(wk_sb, wk_d)
        wv_sb = []
        for ct in range(CT):
            t = consts.tile([P, c_dim], f32, tag=f"wv{ct}")
            nc.sync.dma_start(t, wv_d[ts(ct, P), :])
            wv_sb.append(t)

        bq_sb = consts.tile([da, 1], f32, tag="bq")
        nc.sync.dma_start(bq_sb, bq_d.rearrange("(a o) -> a o", o=1))
        bk_sb = consts.tile([da, 1], f32, tag="bk")
        nc.sync.dma_start(bk_sb, bk_d.rearrange("(a o) -> a o", o=1))
        bv_sb = consts.tile([P, CT], f32, tag="bv")
        nc.sync.dma_start(bv_sb, bv_d.rearrange("(ct p) -> p ct", p=P))

        # PE warmup: dummy matmuls on the identity keep the HAM activity
        # window busy while the first x DMA lands, so real matmuls start at
        # the warm 2.4 GHz clock instead of 1.2 GHz.
        warm_ps = ps_s.tile([P, ic], f32, tag="ps", name="warm_ps")
        for _ in range(24):
            nc.tensor.matmul(
                warm_ps[:, :P], ident, ident, start=True, stop=True
            )

        # Transposed weights via PE transpose: wqT/wkT[kc] = [128, da],
        # wvT[kc] = [128, c_dim] (= Wv[:, kc-cols].T laid out c' x c).
        # Stored bf16 (the PSUM->SBUF copy converts) for full-rate matmuls.
        wqT = consts.tile([P, KC, da], bf16, tag="wqT")
        wkT = consts.tile([P, KC, da], bf16, tag="wkT")
        wvT = consts.tile([P, KC, c_dim], bf16, tag="wvT")
        for kc in range(KC):
            pt = ps_s.tile([P, P], f32, tag="ps")
            nc.tensor.transpose(pt[:, :da], wq_sb[:, ts(kc, P)], ident[:da, :da])
            nc.scalar.copy(wqT[:, kc, :], pt[:, :da])
            pt2 = ps_s.tile([P, P], f32, tag="ps")
            nc.tensor.transpose(pt2[:, :da], wk_sb[:, ts(kc, P)], ident[:da, :da])
            nc.scalar.copy(wkT[:, kc, :], pt2[:, :da])
            for ct in range(CT):
                pt3 = ps_s.tile([P, P], f32, tag="ps")
                nc.tensor.transpose(pt3, wv_sb[ct][:, ts(kc, P)], ident)
                nc.scalar.copy(wvT[:, kc, ts(ct, P)], pt3)

        for b in [b for _ in range(repeat) for b in range(bpc)]:
            # --- phase 1: load x ---
            x_sb = []
            for ct in range(CT):
                t = xpool.tile([P, n_dim], f32, tag=f"x{ct}", name=f"x{ct}")
                for half in range(4):
                    nc.sync.dma_start(
                        t[:, ts(half, n_dim // 4)],
                        x_d[b, ts(ct, P), ts(half, n_dim // 4)],
                    )
                x_sb.append(t)
            # bf16 copy of x feeding the QKV projection matmuls, split into
            # chunks so the first projection matmuls start early
            x_bf = []
            for ct in range(CT):
                t = bigs.tile([P, n_dim], bf16, tag=f"xbf{ct}", name=f"xbf{ct}")
                for n_i in range(NIC):
                    nc.vector.tensor_copy(
                        t[:, ts(n_i, ic)], x_sb[ct][:, ts(n_i, ic)]
                    )
                x_bf.append(t)

            # --- phase 2: q, k [128, n] bf16, replicated into both partition
            # halves so the scores matmuls can be 2-way row-packed (K=64 each
            # at row groups 0 and 64). ---
            q_sb = bigs.tile([P, n_dim], bf16, tag="q")
            k_sb = bigs.tile([P, n_dim], bf16, tag="k")
            for n_i in range(NIC):
                pq = ps_s.tile([da, ic], f32, tag="ps")
                for kc in range(KC):
                    nc.tensor.matmul(
                        pq,
                        wqT[:, kc, :],
                        x_bf[kc][:, ts(n_i, ic)],
                        start=(kc == 0),
                        stop=(kc == KC - 1),
                    )
                nc.vector.tensor_scalar_add(q_sb[:da, ts(n_i, ic)], pq, bq_sb)
                nc.vector.tensor_copy(q_sb[da:, ts(n_i, ic)], q_sb[:da, ts(n_i, ic)])
                pk = ps_s.tile([da, ic], f32, tag="ps")
                for kc in range(KC):
                    nc.tensor.matmul(
                        pk,
                        wkT[:, kc, :],
                        x_bf[kc][:, ts(n_i, ic)],
                        start=(kc == 0),
                        stop=(kc == KC - 1),
                    )
                nc.vector.tensor_scalar_add(k_sb[:da, ts(n_i, ic)], pk, bk_sb)
                nc.vector.tensor_copy(k_sb[da:, ts(n_i, ic)], k_sb[:da, ts(n_i, ic)])

            # --- phase 3: vT [n, c] fp8, stored DoubleRow-ready as
            # [128, NJT/2, 2, c] (middle dims: j-tile pair, pair member) ---
            vT_sb = bigs.tile([P, NJT // 2, 2, c_dim], f8, tag="vT")
            for t_j in range(NJT):
                pv = ps_s.tile([P, c_dim], f32, tag="ps")
                for kc in range(KC):
                    nc.tensor.matmul(
                        pv,
                        x_bf[kc][:, ts(t_j, P)],
                        wvT[:, kc, :],
                        start=(kc == 0),
                        stop=(kc == KC - 1),
                    )
                nc.vector.tensor_copy(vT_sb[:, t_j // 2, t_j % 2, :], pv)

            # --- phase 4: attention main loop ---
            for i_c in range(NIC):
                po = [
                    ps_out.tile([P, ic], f32, tag="o", name=f"po{c0}")
                    for c0 in range(CT)
                ]
                pd = ps_d.tile([P, ic], f32, tag="d")
                # Software-pipelined emission: PE engine queues are strict
                # FIFO, so out-matmuls are emitted one pair behind the score
                # matmuls (hiding the exp latency behind queued PE work) and
                # the denominator matmul two pairs behind (hiding the DVE
                # pair-sum latency).
                NP = NJT // 2
                NQ = NP // 2
                ets = [None] * NP
                esums = [None] * NQ

                def emit_scores(jp):
                    # two K=64 score matmuls packed into row groups 0 / 64,
                    # outputs to the two banks of one [128, 1024] psum tile
                    ps_pair = ps_s.tile([P, 2 * ic], f32, tag="ps", name="ps_pair")
                    nc.tensor.matmul(
                        ps_pair[:, ts(0, ic)],
                        k_sb[:da, ts(2 * jp, P)],
                        q_sb[:da, ts(i_c, ic)],
                        start=True,
                        stop=True,
                        tile_position=(0, 0),
                    )
                    nc.tensor.matmul(
                        ps_pair[:, ts(1, ic)],
                        k_sb[da:, ts(2 * jp + 1, P)],
                        q_sb[da:, ts(i_c, ic)],
                        start=True,
                        stop=True,
                        tile_position=(da, 0),
                    )
                    # exp -> fp8 E^T, already in DoubleRow [P, 2, ic] layout
                    et = et_pool.tile([P, 2 * ic], f8, tag="et", name="et")
                    nc.scalar.activation(
                        et, ps_pair, mybir.ActivationFunctionType.Exp, scale=inv_sqrt_n
                    )
                    ets[jp] = et

                def emit_out(jp):
                    for c0 in range(CT):
                        nc.tensor.matmul(
                            po[c0],
                            vT_sb[:, jp, :, ts(c0, P)],
                            ets[jp].rearrange("p (two f) -> p two f", two=2),
                            start=(jp == 0),
                            stop=(jp == NP - 1),
                            perf_mode=mybir.MatmulPerfMode.DoubleRow,
                            skip_group_check=True,
                        )

                def emit_d(jp):
                    nc.tensor.matmul(
                        pd,
                        ones_f8_v,
                        ets[jp].rearrange("p (two f) -> p two f", two=2),
                        start=(jp == 0),
                        stop=(jp == NP - 1),
                        perf_mode=mybir.MatmulPerfMode.DoubleRow,
                        skip_group_check=True,
                    )

                for jp in range(NP):
                    emit_scores(jp)
                    if jp >= 2:
                        emit_out(jp - 2)
                        emit_d(jp - 2)
                for jp in (NP - 2, NP - 1):
                    emit_out(jp)
                    emit_d(jp)
                # tail: out = po*recipD + bv + x
                rd = small.tile([P, ic], f32, tag="rd")
                nc.vector.reciprocal_approx_fast(rd, pd)
                for c0 in range(CT):
                    ob = outs.tile([P, ic], f32, tag="ob")
                    nc.vector.tensor_mul(ob, po[c0], rd)
                    nc.vector.scalar_tensor_tensor(
                        ob,
                        ob,
                        bv_sb[:, ds(c0, 1)],
                        x_sb[c0][:, ts(i_c, ic)],
                        mybir.AluOpType.add,
                        mybir.AluOpType.add,
                    )
                    nc.sync.dma_start(out_d[b, ts(c0, P), ts(i_c, ic)], ob)

    nc.compile()
    return nc


_NC_CACHE = None


def get_nc():
    global _NC_CACHE
    if _NC_CACHE is None:
        _NC_CACHE = build_nc()
    return _NC_CACHE


def make_in_maps(inputs) -> list:
    x = np.ascontiguousarray(np.asarray(inputs["x"], dtype=np.float32)).reshape(
        B, C, N
    )
    w = {
        name: np.ascontiguousarray(np.asarray(inputs[name], dtype=np.float32))
        for name in ("Wq", "bq", "Wk", "bk", "Wv", "bv")
    }
    in_maps = []
    for c in range(N_CORES):
        m = {"x": np.ascontiguousarray(x[c * BPC : (c + 1) * BPC])}
        m.update(w)
        in_maps.append(m)
    return in_maps


def kernel(**inputs) -> np.ndarray:
    from concourse.bass_utils import run_bass_kernel_spmd

    res = run_bass_kernel_spmd(
        get_nc(), make_in_maps(inputs), core_ids=list(range(N_CORES))
    )
    out = np.concatenate([r["out"] for r in res.results], axis=0)
    return out.reshape(B, C, H, W).astype(np.float32)
